# revision 3
# baseline (speedup 1.0000x reference)
"""Trainium2 Bass kernel for the MDN module (nn_MDN_module_55791625175524).

Strategy: pure data parallel over 8 NeuronCores, batch-major layout on-chip
([128 partitions, C rows] tiles, fp32). The 2->64->4 MLP runs as an unrolled
hidden-unit loop with the per-unit affine folded into engine-immediate scalars:
  g_j = a_j*x0 + b_j*x1  computed as  Relu(scale=coef * (ratio*x_sel + x_oth))
so ReLU costs one ACT op with the multiply for free. Head accumulation is
split across Vector (DVE), Scalar (ACT, Copy-with-scale) and GpSimd engines.
The tail (Lyapunov rescale, rsample, log-prob) is ~30 elementwise ops; 1/var
and sqrt(var) use exp(-lv)/exp(0.5*lv) so only one ACT table set loads.
logp is reduced to per-partition partials on-chip and finished on host.
"""

import numpy as np

BETA = 0.99
EPS_V = 1e-3
LOG_2PI = float(np.log(2.0 * np.pi))

B = 2_097_152
N_CORES = 8
BC = B // N_CORES          # rows per core
C = 512                    # rows per partition per tile
P = 128
TILES = BC // (P * C)      # 4


def _build_program(W1, b1, W2, b2, Wv):
    import concourse.bass as bass
    import concourse.bacc as bacc
    import concourse.tile as tile
    import concourse.mybir as mybir

    dt = mybir.dt.float32
    Alu = mybir.AluOpType
    Act = mybir.ActivationFunctionType

    G = (Wv @ Wv.T).astype(np.float64)
    g00, g01, g11 = float(G[0, 0]), float(G[0, 1]), float(G[1, 1])

    nc = bacc.Bacc(
        "TRN2", target_bir_lowering=False, debug=False, num_devices=N_CORES
    )

    x_d = nc.dram_tensor("x", [BC, 2], dt, kind="ExternalInput").ap()
    y_d = nc.dram_tensor("y", [BC, 2], dt, kind="ExternalInput").ap()
    e_d = nc.dram_tensor("eps", [BC, 2], dt, kind="ExternalInput").ap()
    fx_d = nc.dram_tensor("fx", [BC, 2], dt, kind="ExternalOutput").ap()
    lp_d = nc.dram_tensor("lp", [P, TILES], dt, kind="ExternalOutput").ap()

    xr = x_d.rearrange("(t p n) c -> t p n c", t=TILES, p=P, n=C)
    yr = y_d.rearrange("(t p n) c -> t p n c", t=TILES, p=P, n=C)
    er = e_d.rearrange("(t p n) c -> t p n c", t=TILES, p=P, n=C)
    fr = fx_d.rearrange("(t p n) c -> t p n c", t=TILES, p=P, n=C)

    with tile.TileContext(nc) as tc:
        with (
            tc.tile_pool(name="io", bufs=2) as io,
            tc.tile_pool(name="acc", bufs=2) as accp,
            tc.tile_pool(name="scr", bufs=1) as scr,
            tc.tile_pool(name="lp", bufs=1) as lpp,
        ):
            LP = lpp.tile([P, TILES], dt)
            for t in range(TILES):
                X = io.tile([P, C, 2], dt, tag="X")
                Y = io.tile([P, C, 2], dt, tag="Y")
                E = io.tile([P, C, 2], dt, tag="E")
                FX = io.tile([P, C, 2], dt, tag="FX")
                nc.sync.dma_start(X[:], xr[t])
                nc.sync.dma_start(Y[:], yr[t])
                nc.sync.dma_start(E[:], er[t])
                x0, x1 = X[:, :, 0:1], X[:, :, 1:2]
                y0, y1 = Y[:, :, 0:1], Y[:, :, 1:2]
                e0, e1 = E[:, :, 0:1], E[:, :, 1:2]

                F = [
                    accp.tile([P, C, 1], dt, tag=f"F{c}", name=f"F{c}")
                    for c in range(4)
                ]

                # ---- MLP: f = relu(x@W1 + b1) @ W2 + b2, unrolled over 64 units
                for j in range(64):
                    aj, bj = float(W1[0, j]), float(W1[1, j])
                    b1j = float(b1[j])
                    tj = scr.tile([P, C, 1], dt, tag="tj")
                    zj = scr.tile([P, C, 1], dt, tag="zj")
                    rj = scr.tile([P, C, 1], dt, tag="rj")
                    if abs(bj) >= abs(aj):
                        xs, xo, coef, ratio = x0, x1, bj, aj / bj
                    else:
                        xs, xo, coef, ratio = x1, x0, aj, bj / aj
                    # t = ratio * xs  (ACT Copy w/ scale); z = t + xo (DVE);
                    # r = relu(coef * z + b1j) (ACT)
                    nc.scalar.activation(tj[:], xs, Act.Copy, bias=0.0, scale=ratio)
                    nc.vector.tensor_tensor(zj[:], tj[:], xo, Alu.add)
                    nc.scalar.activation(
                        rj[:], zj[:], Act.Relu, bias=b1j, scale=coef
                    )
                    w2j = [float(W2[j, c]) for c in range(4)]
                    if j == 0:
                        for c in range(4):
                            eng = nc.gpsimd if c == 3 else nc.vector
                            eng.tensor_scalar(
                                F[c][:], rj[:], w2j[c], float(b2[c]),
                                Alu.mult, Alu.add,
                            )
                    else:
                        tm0 = scr.tile([P, C, 1], dt, tag="tm0")
                        tm1 = scr.tile([P, C, 1], dt, tag="tm1")
                        tm2 = scr.tile([P, C, 1], dt, tag="tm2")
                        tm3 = scr.tile([P, C, 1], dt, tag="tm3")
                        nc.vector.tensor_scalar(
                            tm0[:], rj[:], w2j[0], None, Alu.mult
                        )
                        nc.scalar.activation(
                            tm1[:], rj[:], Act.Copy, bias=0.0, scale=w2j[1]
                        )
                        nc.scalar.activation(
                            tm2[:], rj[:], Act.Copy, bias=0.0, scale=w2j[2]
                        )
                        nc.gpsimd.tensor_scalar(
                            tm3[:], rj[:], w2j[3], None, Alu.mult
                        )
                        nc.vector.tensor_tensor(F[0][:], F[0][:], tm0[:], Alu.add)
                        nc.vector.tensor_tensor(F[1][:], F[1][:], tm1[:], Alu.add)
                        nc.vector.tensor_tensor(F[2][:], F[2][:], tm2[:], Alu.add)
                        nc.gpsimd.tensor_tensor(F[3][:], F[3][:], tm3[:], Alu.add)
                mu0, mu1, lv0, lv1 = F

                def quad(out_tag, u0, u1):
                    # u.G.u + EPS_V via G-quadratic form
                    s0 = scr.tile([P, C, 1], dt, tag=out_tag + "s0")
                    s1 = scr.tile([P, C, 1], dt, tag=out_tag + "s1")
                    pp = scr.tile([P, C, 1], dt, tag=out_tag + "pp")
                    va = scr.tile([P, C, 1], dt, tag=out_tag + "va")
                    vb = scr.tile([P, C, 1], dt, tag=out_tag + "vb")
                    vc = scr.tile([P, C, 1], dt, tag=out_tag + "vc")
                    vo = scr.tile([P, C, 1], dt, tag=out_tag + "vo")
                    nc.scalar.activation(s0[:], u0, Act.Square, bias=0.0)
                    nc.scalar.activation(s1[:], u1, Act.Square, bias=0.0)
                    nc.vector.tensor_tensor(pp[:], u0, u1, Alu.mult)
                    nc.vector.tensor_scalar(va[:], s0[:], g00, None, Alu.mult)
                    nc.vector.tensor_scalar(vb[:], pp[:], 2.0 * g01, None, Alu.mult)
                    nc.vector.tensor_scalar(
                        vc[:], s1[:], g11, EPS_V, Alu.mult, Alu.add
                    )
                    nc.vector.tensor_tensor(va[:], va[:], vb[:], Alu.add)
                    nc.vector.tensor_tensor(vo[:], va[:], vc[:], Alu.add)
                    return vo

                vx = quad("vx", x0, x1)
                vm = quad("vm", mu0[:], mu1[:])

                rv = scr.tile([P, C, 1], dt, tag="rv")
                sc = scr.tile([P, C, 1], dt, tag="sc")
                sc2 = scr.tile([P, C, 1], dt, tag="sc2")
                nc.vector.reciprocal(rv[:], vm[:])
                nc.vector.tensor_tensor(sc[:], vx[:], rv[:], Alu.mult)
                # scale = min(BETA*vx/vm, 1)  ==  reference's relu clamp form
                nc.vector.tensor_scalar(
                    sc2[:], sc[:], BETA, 1.0, Alu.mult, Alu.min
                )
                ms0 = scr.tile([P, C, 1], dt, tag="ms0")
                ms1 = scr.tile([P, C, 1], dt, tag="ms1")
                nc.vector.tensor_tensor(ms0[:], mu0[:], sc2[:], Alu.mult)
                nc.vector.tensor_tensor(ms1[:], mu1[:], sc2[:], Alu.mult)

                sd0 = scr.tile([P, C, 1], dt, tag="sd0")
                sd1 = scr.tile([P, C, 1], dt, tag="sd1")
                iv0 = scr.tile([P, C, 1], dt, tag="iv0")
                iv1 = scr.tile([P, C, 1], dt, tag="iv1")
                nc.scalar.activation(sd0[:], lv0[:], Act.Exp, scale=0.5)
                nc.scalar.activation(sd1[:], lv1[:], Act.Exp, scale=0.5)
                nc.scalar.activation(iv0[:], lv0[:], Act.Exp, scale=-1.0)
                nc.scalar.activation(iv1[:], lv1[:], Act.Exp, scale=-1.0)

                # fx = mu_stable + sqrt(var)*eps
                f0 = scr.tile([P, C, 1], dt, tag="f0")
                f1 = scr.tile([P, C, 1], dt, tag="f1")
                nc.vector.tensor_tensor(f0[:], sd0[:], e0, Alu.mult)
                nc.vector.tensor_tensor(f1[:], sd1[:], e1, Alu.mult)
                nc.vector.tensor_tensor(FX[:, :, 0:1], f0[:], ms0[:], Alu.add)
                nc.vector.tensor_tensor(FX[:, :, 1:2], f1[:], ms1[:], Alu.add)
                nc.sync.dma_start(fr[t], FX[:])

                # s = d0^2/var0 + d1^2/var1 + lv0 + lv1 ; partial-sum per partition
                d0 = scr.tile([P, C, 1], dt, tag="d0")
                d1 = scr.tile([P, C, 1], dt, tag="d1")
                q0 = scr.tile([P, C, 1], dt, tag="q0")
                q1 = scr.tile([P, C, 1], dt, tag="q1")
                u1t = scr.tile([P, C, 1], dt, tag="u1t")
                u2t = scr.tile([P, C, 1], dt, tag="u2t")
                u3t = scr.tile([P, C, 1], dt, tag="u3t")
                dumm = scr.tile([P, C, 1], dt, tag="dumm")
                nc.vector.tensor_tensor(d0[:], y0, ms0[:], Alu.subtract)
                nc.vector.tensor_tensor(d1[:], y1, ms1[:], Alu.subtract)
                nc.scalar.activation(q0[:], d0[:], Act.Square, bias=0.0)
                nc.scalar.activation(q1[:], d1[:], Act.Square, bias=0.0)
                nc.vector.tensor_tensor(q0[:], q0[:], iv0[:], Alu.mult)
                nc.vector.tensor_tensor(q1[:], q1[:], iv1[:], Alu.mult)
                nc.vector.tensor_tensor(u1t[:], q0[:], q1[:], Alu.add)
                nc.vector.tensor_tensor(u2t[:], lv0[:], lv1[:], Alu.add)
                nc.vector.tensor_tensor(u3t[:], u1t[:], u2t[:], Alu.add)
                nc.scalar.activation(
                    dumm[:], u3t[:], Act.Copy, bias=0.0,
                    accum_out=LP[:, t : t + 1],
                )
            nc.sync.dma_start(lp_d, LP[:])
    nc.compile()
    return nc


def kernel(x, y, eps, W1, b1, W2, b2, Wv):
    x = np.ascontiguousarray(np.asarray(x, dtype=np.float32))
    y2 = np.ascontiguousarray(np.asarray(y, dtype=np.float32).reshape(-1, 2))
    e2 = np.ascontiguousarray(np.asarray(eps, dtype=np.float32).reshape(-1, 2))
    W1 = np.asarray(W1, dtype=np.float32)
    b1 = np.asarray(b1, dtype=np.float32)
    W2 = np.asarray(W2, dtype=np.float32)
    b2 = np.asarray(b2, dtype=np.float32)
    Wv = np.asarray(Wv, dtype=np.float32)

    from concourse.bass_utils import run_bass_kernel_spmd

    nc = _build_program(W1, b1, W2, b2, Wv)

    in_maps = []
    for c in range(N_CORES):
        sl = slice(c * BC, (c + 1) * BC)
        in_maps.append({"x": x[sl], "y": y2[sl], "eps": e2[sl]})

    res = run_bass_kernel_spmd(nc, in_maps, core_ids=list(range(N_CORES)))
    outs = res.results

    fx = np.concatenate([outs[c]["fx"] for c in range(N_CORES)], axis=0)
    fx = fx.reshape(-1, 1, 2).astype(np.float32)
    s_total = sum(float(outs[c]["lp"].astype(np.float64).sum())
                  for c in range(N_CORES))
    logp_y = np.float32(0.5 * s_total + B * LOG_2PI)
    return fx, logp_y


# revision 7
# speedup vs baseline: 1.0761x; 1.0761x over previous
"""Trainium2 Bass kernel for the MDN module (nn_MDN_module_55791625175524).

Strategy: pure data parallel over 8 NeuronCores, batch-major layout on-chip
([128 partitions, C rows] tiles, fp32). The 2->64->4 MLP runs as an unrolled
hidden-unit loop with the per-unit affine folded into engine-immediate scalars:
  g_j = a_j*x0 + b_j*x1  computed as  Relu(scale=coef * (ratio*x_sel + x_oth))
so ReLU costs one ACT op with the multiply for free. Head accumulation is
split across Vector (DVE), Scalar (ACT, Copy-with-scale) and GpSimd engines.
The tail (Lyapunov rescale, rsample, log-prob) is ~30 elementwise ops; 1/var
and sqrt(var) use exp(-lv)/exp(0.5*lv) so only one ACT table set loads.
logp is reduced to per-partition partials on-chip and finished on host.
"""

import numpy as np

BETA = 0.99
EPS_V = 1e-3
LOG_2PI = float(np.log(2.0 * np.pi))

B = 2_097_152
N_CORES = 8
BC = B // N_CORES          # rows per core
C = 1024                   # rows per partition per MLP tile
CH = 512                   # rows per partition per tail sub-tile
P = 128
TILES = BC // (P * C)      # 2
HALVES = C // CH           # 2


def _build_program(W1, b1, W2, b2, Wv):
    import concourse.bass as bass
    import concourse.bacc as bacc
    import concourse.tile as tile
    import concourse.mybir as mybir

    dt = mybir.dt.float32
    Alu = mybir.AluOpType
    Act = mybir.ActivationFunctionType

    G = (Wv @ Wv.T).astype(np.float64)
    g00, g01, g11 = float(G[0, 0]), float(G[0, 1]), float(G[1, 1])

    nc = bacc.Bacc(
        "TRN2", target_bir_lowering=False, debug=False, num_devices=N_CORES
    )

    x_d = nc.dram_tensor("x", [BC, 2], dt, kind="ExternalInput").ap()
    y_d = nc.dram_tensor("y", [BC, 2], dt, kind="ExternalInput").ap()
    e_d = nc.dram_tensor("eps", [BC, 2], dt, kind="ExternalInput").ap()
    fx_d = nc.dram_tensor("fx", [BC, 2], dt, kind="ExternalOutput").ap()
    lp_d = nc.dram_tensor(
        "lp", [P, TILES * HALVES], dt, kind="ExternalOutput"
    ).ap()

    xr = x_d.rearrange("(t p n) c -> t p n c", t=TILES, p=P, n=C)
    yr = y_d.rearrange("(t p n) c -> t p n c", t=TILES, p=P, n=C)
    er = e_d.rearrange("(t p n) c -> t p n c", t=TILES, p=P, n=C)
    fr = fx_d.rearrange("(t p n) c -> t p n c", t=TILES, p=P, n=C)

    with tile.TileContext(nc) as tc:
        with (
            tc.tile_pool(name="io", bufs=2) as io,
            tc.tile_pool(name="acc", bufs=2) as accp,
            tc.tile_pool(name="scr", bufs=1) as scr,
            tc.tile_pool(name="lp", bufs=1) as lpp,
        ):
            LP = lpp.tile([P, TILES * HALVES], dt)
            for t in range(TILES):
                X = io.tile([P, C, 2], dt, tag="X")
                Y = io.tile([P, C, 2], dt, tag="Y")
                E = io.tile([P, C, 2], dt, tag="E")
                FX = io.tile([P, C, 2], dt, tag="FX")
                nc.sync.dma_start(X[:], xr[t])
                nc.sync.dma_start(Y[:], yr[t])
                nc.sync.dma_start(E[:], er[t])
                x0, x1 = X[:, :, 0:1], X[:, :, 1:2]
                y0, y1 = Y[:, :, 0:1], Y[:, :, 1:2]
                e0, e1 = E[:, :, 0:1], E[:, :, 1:2]

                F = [
                    accp.tile([P, C, 1], dt, tag=f"F{c}", name=f"F{c}")
                    for c in range(4)
                ]

                # ---- MLP: f = relu(x@W1 + b1) @ W2 + b2, unrolled over 64 units
                for j in range(64):
                    aj, bj = float(W1[0, j]), float(W1[1, j])
                    b1j = float(b1[j])
                    tj = scr.tile([P, C, 1], dt, tag="tj")
                    zj = scr.tile([P, C, 1], dt, tag="zj")
                    rj = scr.tile([P, C, 1], dt, tag="rj")
                    if abs(bj) >= abs(aj):
                        xs, xo, coef, ratio = x0, x1, bj, aj / bj
                    else:
                        xs, xo, coef, ratio = x1, x0, aj, bj / aj
                    # t = ratio * xs  (ACT Copy w/ scale); z = t + xo (DVE);
                    # r = relu(coef * z + b1j) (ACT)
                    nc.scalar.activation(tj[:], xs, Act.Copy, bias=0.0, scale=ratio)
                    nc.vector.tensor_tensor(zj[:], tj[:], xo, Alu.add)
                    nc.scalar.activation(
                        rj[:], zj[:], Act.Relu, bias=b1j, scale=coef
                    )
                    w2j = [float(W2[j, c]) for c in range(4)]
                    if j == 0:
                        for c in range(4):
                            eng = nc.gpsimd if c == 3 else nc.vector
                            eng.tensor_scalar(
                                F[c][:], rj[:], w2j[c], float(b2[c]),
                                Alu.mult, Alu.add,
                            )
                    else:
                        tm0 = scr.tile([P, C, 1], dt, tag="tm0")
                        tm1 = scr.tile([P, C, 1], dt, tag="tm1")
                        tm2 = scr.tile([P, C, 1], dt, tag="tm2")
                        tm3 = scr.tile([P, C, 1], dt, tag="tm3")
                        nc.vector.tensor_scalar(
                            tm0[:], rj[:], w2j[0], None, Alu.mult
                        )
                        nc.scalar.activation(
                            tm1[:], rj[:], Act.Copy, bias=0.0, scale=w2j[1]
                        )
                        nc.scalar.activation(
                            tm2[:], rj[:], Act.Copy, bias=0.0, scale=w2j[2]
                        )
                        nc.gpsimd.tensor_scalar(
                            tm3[:], rj[:], w2j[3], None, Alu.mult
                        )
                        nc.vector.tensor_tensor(F[0][:], F[0][:], tm0[:], Alu.add)
                        nc.vector.tensor_tensor(F[1][:], F[1][:], tm1[:], Alu.add)
                        nc.vector.tensor_tensor(F[2][:], F[2][:], tm2[:], Alu.add)
                        nc.gpsimd.tensor_tensor(F[3][:], F[3][:], tm3[:], Alu.add)
                def quad(out_tag, u0, u1):
                    # u.G.u + EPS_V via G-quadratic form
                    s0 = scr.tile([P, CH, 1], dt, tag=out_tag + "s0")
                    s1 = scr.tile([P, CH, 1], dt, tag=out_tag + "s1")
                    pp = scr.tile([P, CH, 1], dt, tag=out_tag + "pp")
                    va = scr.tile([P, CH, 1], dt, tag=out_tag + "va")
                    vb = scr.tile([P, CH, 1], dt, tag=out_tag + "vb")
                    vc = scr.tile([P, CH, 1], dt, tag=out_tag + "vc")
                    vo = scr.tile([P, CH, 1], dt, tag=out_tag + "vo")
                    nc.scalar.activation(s0[:], u0, Act.Square, bias=0.0)
                    nc.scalar.activation(s1[:], u1, Act.Square, bias=0.0)
                    nc.vector.tensor_tensor(pp[:], u0, u1, Alu.mult)
                    nc.vector.tensor_scalar(va[:], s0[:], g00, None, Alu.mult)
                    nc.vector.tensor_scalar(vb[:], pp[:], 2.0 * g01, None, Alu.mult)
                    nc.vector.tensor_scalar(
                        vc[:], s1[:], g11, EPS_V, Alu.mult, Alu.add
                    )
                    nc.vector.tensor_tensor(va[:], va[:], vb[:], Alu.add)
                    nc.vector.tensor_tensor(vo[:], va[:], vc[:], Alu.add)
                    return vo

                for h in range(HALVES):
                    hs = slice(h * CH, (h + 1) * CH)
                    x0h, x1h = X[:, hs, 0:1], X[:, hs, 1:2]
                    y0h, y1h = Y[:, hs, 0:1], Y[:, hs, 1:2]
                    e0h, e1h = E[:, hs, 0:1], E[:, hs, 1:2]
                    mu0, mu1 = F[0][:, hs, :], F[1][:, hs, :]
                    lv0, lv1 = F[2][:, hs, :], F[3][:, hs, :]

                    vx = quad("vx", x0h, x1h)
                    vm = quad("vm", mu0, mu1)

                    rv = scr.tile([P, CH, 1], dt, tag="rv")
                    sc = scr.tile([P, CH, 1], dt, tag="sc")
                    sc2 = scr.tile([P, CH, 1], dt, tag="sc2")
                    nc.vector.reciprocal(rv[:], vm[:])
                    nc.vector.tensor_tensor(sc[:], vx[:], rv[:], Alu.mult)
                    # scale = min(BETA*vx/vm, 1)  ==  reference's relu clamp
                    nc.vector.tensor_scalar(
                        sc2[:], sc[:], BETA, 1.0, Alu.mult, Alu.min
                    )
                    ms0 = scr.tile([P, CH, 1], dt, tag="ms0")
                    ms1 = scr.tile([P, CH, 1], dt, tag="ms1")
                    nc.vector.tensor_tensor(ms0[:], mu0, sc2[:], Alu.mult)
                    nc.vector.tensor_tensor(ms1[:], mu1, sc2[:], Alu.mult)

                    sd0 = scr.tile([P, CH, 1], dt, tag="sd0")
                    sd1 = scr.tile([P, CH, 1], dt, tag="sd1")
                    iv0 = scr.tile([P, CH, 1], dt, tag="iv0")
                    iv1 = scr.tile([P, CH, 1], dt, tag="iv1")
                    nc.scalar.activation(sd0[:], lv0, Act.Exp, scale=0.5)
                    nc.scalar.activation(sd1[:], lv1, Act.Exp, scale=0.5)
                    nc.scalar.activation(iv0[:], lv0, Act.Exp, scale=-1.0)
                    nc.scalar.activation(iv1[:], lv1, Act.Exp, scale=-1.0)

                    # fx = mu_stable + sqrt(var)*eps
                    f0 = scr.tile([P, CH, 1], dt, tag="f0")
                    f1 = scr.tile([P, CH, 1], dt, tag="f1")
                    nc.vector.tensor_tensor(f0[:], sd0[:], e0h, Alu.mult)
                    nc.vector.tensor_tensor(f1[:], sd1[:], e1h, Alu.mult)
                    nc.vector.tensor_tensor(
                        FX[:, hs, 0:1], f0[:], ms0[:], Alu.add
                    )
                    nc.vector.tensor_tensor(
                        FX[:, hs, 1:2], f1[:], ms1[:], Alu.add
                    )

                    # s = d0^2/var0 + d1^2/var1 + lv0 + lv1; per-partition sum
                    d0 = scr.tile([P, CH, 1], dt, tag="d0")
                    d1 = scr.tile([P, CH, 1], dt, tag="d1")
                    q0 = scr.tile([P, CH, 1], dt, tag="q0")
                    q1 = scr.tile([P, CH, 1], dt, tag="q1")
                    u1t = scr.tile([P, CH, 1], dt, tag="u1t")
                    u2t = scr.tile([P, CH, 1], dt, tag="u2t")
                    u3t = scr.tile([P, CH, 1], dt, tag="u3t")
                    dumm = scr.tile([P, CH, 1], dt, tag="dumm")
                    nc.vector.tensor_tensor(d0[:], y0h, ms0[:], Alu.subtract)
                    nc.vector.tensor_tensor(d1[:], y1h, ms1[:], Alu.subtract)
                    nc.scalar.activation(q0[:], d0[:], Act.Square, bias=0.0)
                    nc.scalar.activation(q1[:], d1[:], Act.Square, bias=0.0)
                    nc.vector.tensor_tensor(q0[:], q0[:], iv0[:], Alu.mult)
                    nc.vector.tensor_tensor(q1[:], q1[:], iv1[:], Alu.mult)
                    nc.vector.tensor_tensor(u1t[:], q0[:], q1[:], Alu.add)
                    nc.vector.tensor_tensor(u2t[:], lv0, lv1, Alu.add)
                    nc.vector.tensor_tensor(u3t[:], u1t[:], u2t[:], Alu.add)
                    nc.scalar.activation(
                        dumm[:], u3t[:], Act.Copy, bias=0.0,
                        accum_out=LP[:, t * HALVES + h : t * HALVES + h + 1],
                    )
                nc.sync.dma_start(fr[t], FX[:])
            nc.sync.dma_start(lp_d, LP[:])
    nc.compile()
    return nc


def kernel(x, y, eps, W1, b1, W2, b2, Wv):
    x = np.ascontiguousarray(np.asarray(x, dtype=np.float32))
    y2 = np.ascontiguousarray(np.asarray(y, dtype=np.float32).reshape(-1, 2))
    e2 = np.ascontiguousarray(np.asarray(eps, dtype=np.float32).reshape(-1, 2))
    W1 = np.asarray(W1, dtype=np.float32)
    b1 = np.asarray(b1, dtype=np.float32)
    W2 = np.asarray(W2, dtype=np.float32)
    b2 = np.asarray(b2, dtype=np.float32)
    Wv = np.asarray(Wv, dtype=np.float32)

    from concourse.bass_utils import run_bass_kernel_spmd

    nc = _build_program(W1, b1, W2, b2, Wv)

    in_maps = []
    for c in range(N_CORES):
        sl = slice(c * BC, (c + 1) * BC)
        in_maps.append({"x": x[sl], "y": y2[sl], "eps": e2[sl]})

    res = run_bass_kernel_spmd(nc, in_maps, core_ids=list(range(N_CORES)))
    outs = res.results

    fx = np.concatenate([outs[c]["fx"] for c in range(N_CORES)], axis=0)
    fx = fx.reshape(-1, 1, 2).astype(np.float32)
    s_total = sum(float(outs[c]["lp"].astype(np.float64).sum())
                  for c in range(N_CORES))
    logp_y = np.float32(0.5 * s_total + B * LOG_2PI)
    return fx, logp_y
